# revision 47
# baseline (speedup 1.0000x reference)
"""Contrastive loss (GRACE-style semi_loss pair) on 8 trn2 NeuronCores.

Math (reference):
    a = z1 / ||z1||_row ; b = z2 / ||z2||_row         (N=8192, D=512)
    refl    = exp(a @ a.T / tau) ; between = exp(a @ b.T / tau)
    l1_i = -log(between_ii / (refl.sum(1) + between.sum(1) - refl_ii))
    l2   = same with (z2, z1) swapped
    loss = mean(0.5 * (l1 + l2))

Identities:
  - between2 (for l2) = between.T -> its row sums are COLUMN sums of
    exp(a@b.T/tau).
  - exp(a@a.T) and exp(b@b.T) are symmetric -> their row sums are also
    column sums.  All three column-sum families ride ONE ReduceScatter
    with a [core: ab|aa|bb] interleaved layout; no ACT accumulators or
    DVE row-reduces needed for aa/bb.
  - refl_ii = exp(1/tau) exactly; dab_i = a_i . b_i from fp8 diag blocks.
  - row sumsq (for 1/norm) = diag of the raw z Gram matrix, computed with
    fp8 DoubleRow diag blocks from a casting-DMA fp8 copy of z -- lands
    directly in [128, blocks] layout for a cheap 2-step Newton rsqrt.

Implementation (v4): single pass over zT; fp8e4 DoubleRow matmuls
(K=256/instr, 2x bf16 rate); aa|bb share one 2-bank PSUM tile and a
single [128,1024] exp.  Column sums: exp(ab) via delayed DVE adds,
exp(aa)/exp(bb) via delayed PE ones-matmul folds (one chunk behind so
neither engine waits on ACT).  Prep is stage-pipelined 2-3 units ahead.
Sharding: data-parallel rows; pinned fp8 stationary for the core's 1024
rows, all 16 512-col chunks streamed as moving operands.
"""

import os

# small collectives: RDH has a ~60-120us latency floor here; Mesh is ~10us.
os.environ.setdefault("NEURON_RT_DBG_RDH_CC", "0")

import numpy as np
from contextlib import ExitStack

KDEBUG = bool(os.environ.get("KDEBUG"))

import concourse.bass as bass
import concourse.tile as tile
from concourse import bacc, mybir
from concourse.bass_utils import run_bass_kernel_spmd

N = 8192
D = 512
P = 128
NCORES = 8
LOCAL = N // NCORES            # 1024 rows per core
M_CH = LOCAL // P              # 8 local row blocks of 128
N_UNITS = 8                    # 1024-column units
N_CH = 16                      # 512-column chunks
KC = D // P                    # 4 contraction chunks of 128
TAU = 0.4
SC = 16.0                      # fp8 operand scale: a~N(0,1/512) -> sigma .71
ESC = 1.0 / (SC * SC * TAU)    # exp() scale folding fp8 scaling + 1/tau
ISC2 = 1.0 / (SC * SC)
EXPD = float(np.exp(1.0 / TAU))
Y0 = float(D) ** -0.5          # Newton rsqrt seed; sumsq ~ 512 +- 6%

FP32 = mybir.dt.float32
BF16 = mybir.dt.bfloat16
FP16 = mybir.dt.float16
FP8 = mybir.dt.float8e4
ALU = mybir.AluOpType
ACTF = mybir.ActivationFunctionType
DR = mybir.MatmulPerfMode.DoubleRow
X_AX = mybir.AxisListType.X


def _build():
    nc = bacc.Bacc("TRN2", debug=False, num_devices=NCORES)
    # fp8 copies are prepared host-side so every device load is a plain
    # (non-cast) DMA; the operand pipeline quantizes to fp8 anyway, so this
    # only stacks a second tiny rounding on the same quantization step.
    z1T = nc.dram_tensor("z1T", [D, N], FP8, kind="ExternalInput").ap()
    z2T = nc.dram_tensor("z2T", [D, N], FP8, kind="ExternalInput").ap()
    z1lT = nc.dram_tensor("z1lT", [D, LOCAL], FP8, kind="ExternalInput").ap()
    z2lT = nc.dram_tensor("z2lT", [D, LOCAL], FP8, kind="ExternalInput").ap()
    eye = nc.dram_tensor("eye", [P, P], FP16, kind="ExternalInput").ap()
    # per-core one-hot row mask: rowmask[p, s*M_CH+m] = 1 iff slot s == core id
    rowmask = nc.dram_tensor("rowmask", [P, N // P], FP32, kind="ExternalInput").ap()
    loss = nc.dram_tensor("loss", [1, 1], FP32, kind="ExternalOutput").ap()
    if KDEBUG:
        dbg = {
            nm: nc.dram_tensor(f"dbg_{nm}", [P, N // P], FP32, kind="ExternalOutput").ap()
            for nm in ("cs_ab", "cs_aa", "cs_bb", "dab", "rs_ab", "d1", "d2")
        }

    with tile.TileContext(nc) as tc, ExitStack() as ctx:
        big = ctx.enter_context(tc.tile_pool(name="big", bufs=1))
        zst = ctx.enter_context(tc.tile_pool(name="zst", bufs=3))
        zqt = ctx.enter_context(tc.tile_pool(name="zqt", bufs=2))
        atp = ctx.enter_context(tc.tile_pool(name="atp", bufs=3))
        small = ctx.enter_context(tc.tile_pool(name="small", bufs=1))
        scratch = ctx.enter_context(tc.tile_pool(name="scratch", bufs=2))
        exa_pool = ctx.enter_context(tc.tile_pool(name="exa_pool", bufs=10))
        exp_pool = ctx.enter_context(tc.tile_pool(name="exp_pool", bufs=10))
        pa2 = ctx.enter_context(tc.tile_pool(name="pa2", bufs=2, space="PSUM"))
        pab = ctx.enter_context(tc.tile_pool(name="pab", bufs=2, space="PSUM"))
        psm = ctx.enter_context(tc.tile_pool(name="psm", bufs=2, space="PSUM"))
        dram = ctx.enter_context(tc.tile_pool(name="dram", bufs=1, space="DRAM"))

        # ---- constants --------------------------------------------------
        # DR stationary needs k-step %16==0 -> 16 ones columns (row 0 used)
        ones_f8 = small.tile([P, 2, 16], FP8, tag="ones_f8", name="ones_f8")
        nc.vector.memset(ones_f8, 1.0)
        ones_f32 = small.tile([P, 1], FP32, tag="ones_f32", name="ones_f32")
        nc.vector.memset(ones_f32, 1.0)
        eye_sb = small.tile([P, P], FP16, tag="eye", name="eye_sb")
        nc.sync.dma_start(out=eye_sb, in_=eye)

        # ---- persistent -------------------------------------------------
        dab = small.tile([P, M_CH], FP32, tag="dab", name="dab")
        rsp_ab = [
            small.tile([P, N_CH], FP32, tag=f"rsp_ab{m}", name=f"rsp_ab{m}")
            for m in range(M_CH)
        ]
        dtrash = small.tile([P, P], BF16, tag="dtrash", name="dtrash")

        # fused AllReduce, bf16, split in two so the bulk rides under the
        # last two compute chunks and only a small AR sits on the tail:
        #  cc1 [4, N]: colsum exp(ab|aa|bb) chunks 0..13 + dab (masked rows)
        #  cc2 [2, N]: row0 = colsum chunks 14,15 packed [r*1024 + (n-14)*512],
        #              row1 = rowsum exp(ab) (masked rows)
        # After both ARs every core holds all global sums and computes the
        # full scalar loss redundantly -> no trailing scalar collective.
        cc1_in = dram.tile([3, N], BF16, tag="cc1_in", name="cc1_in")
        cc1_out = dram.tile(
            [3, N], BF16, tag="cc1_out", name="cc1_out", addr_space="Shared"
        )
        cc2_in = dram.tile([2, N], BF16, tag="cc2_in", name="cc2_in")
        cc2_out = dram.tile(
            [2, N], BF16, tag="cc2_out", name="cc2_out", addr_space="Shared"
        )
        mask_sb = small.tile([P, N // P], FP32, tag="mask_sb", name="mask_sb")
        nc.sync.dma_start(out=mask_sb, in_=rowmask)

        # ---- unit prep (staged) ----------------------------------------
        # s1: casting loads (bf16 + raw fp8), Gram-diag sumsq
        # s2: Newton rsqrt, broadcast round-trip
        # s3: fp8 operand scaling
        class Prep:
            pass

        def load_sb(src1, src2, width, name):
            """One fp8 superblock load per tensor: [P, KC, width] with
            width*1B descriptors per (p, k) line -- packet-rate friendly."""
            t1 = zst.tile([P, KC, width], FP8, tag=f"sb1_{name}",
                          name=f"sb1_{name}", bufs=1)
            nc.sync.dma_start(
                out=t1, in_=src1.rearrange("(k p) j -> p k j", p=P)
            )
            t2 = zst.tile([P, KC, width], FP8, tag=f"sb2_{name}",
                          name=f"sb2_{name}", bufs=1)
            nc.scalar.dma_start(
                out=t2, in_=src2.rearrange("(k p) j -> p k j", p=P)
            )
            return t1, t2

        def prep_s1(zs1, zs2, name):
            p_ = Prep()
            p_.name = name
            p_.zs1 = zs1
            p_.zs2 = zs2
            # column sumsq via ones-DR-matmul over an fp8 squared copy, then
            # ACT Rsqrt straight out of PSUM (SC fold: rsqrt(ss/SC^2)=SC/√ss)
            p_.rl = scratch.tile([1, 2048], BF16, tag="rl", name=f"rl_{name}",
                                 bufs=3)
            # rsqrt via exp(-0.5*ln(x)) with exactly ONE Ln and ONE Exp
            # per unit: Ln and Exp live in different ACT tables, and every
            # Ln instruction costs two table swaps around it -- so the four
            # sumsq quarters are first gathered into one staging tile.
            stg = scratch.tile(
                [1, 2048], BF16, tag="stg", name=f"stg_{name}", bufs=2
            )
            for t, zs in ((0, p_.zs1), (1, p_.zs2)):
                zq = zqt.tile([P, KC, 1024], FP8, tag="zq", name=f"zq_{name}_{t}")
                eng = nc.vector if t == 0 else nc.gpsimd
                eng.tensor_mul(zq, zs, zs)
                for hf in range(2):
                    sq = psm.tile([16, D], FP32, tag="ps_small",
                                  name=f"sq_{name}_{t}_{hf}")
                    for kp in range(2):
                        nc.tensor.matmul(
                            sq, ones_f8,
                            zq[:, 2 * kp : 2 * kp + 2, 512 * hf : 512 * hf + 512],
                            start=(kp == 0), stop=(kp == 1), perf_mode=DR,
                        )
                    o = 1024 * t + 512 * hf
                    nc.vector.tensor_copy(stg[:, o : o + 512], sq[0:1, :])
            lnt = scratch.tile(
                [1, 2048], FP16, tag="lnt", name=f"lnt_{name}", bufs=2
            )
            nc.scalar.activation(
                out=lnt, in_=stg, func=ACTF.Ln, scale=1.0 / (SC * SC)
            )
            nc.scalar.activation(
                out=p_.rl, in_=lnt, func=ACTF.Exp, scale=-0.5
            )
            return p_

        def prep_s2(p_):
            name = p_.name
            rl_d = dram.tile([1, 2048], BF16, tag=f"rld_{name}", name=f"rld_{name}")
            nc.gpsimd.dma_start(out=rl_d, in_=p_.rl)
            rb = scratch.tile(
                [P, 2048], BF16, tag="rb", name=f"rb_{name}", bufs=3
            )
            nc.gpsimd.dma_start(
                out=rb, in_=rl_d.to_broadcast([P, 2048])
            )
            p_.rb1 = rb[:, 0:1024]
            p_.rb2 = rb[:, 1024:2048]

        def prep_s3(p_, pin=False):
            name = p_.name
            if pin:
                at1 = big.tile([P, KC, 1024], FP8, tag="ATL1", name="ATL1")
                at2 = big.tile([P, KC, 1024], FP8, tag="ATL2", name="ATL2")
            else:
                at1 = atp.tile([P, KC, 1024], FP8, tag="at1", name=f"at1_{name}")
                at2 = atp.tile([P, KC, 1024], FP8, tag="at2", name=f"at2_{name}")
            for k in range(KC):
                eng = nc.gpsimd if k == 0 else nc.vector
                eng.tensor_mul(at1[:, k, :], p_.zs1[:, k, :], p_.rb1)
                eng = nc.gpsimd if k == 3 else nc.vector
                eng.tensor_mul(at2[:, k, :], p_.zs2[:, k, :], p_.rb2)
            return at1, at2

        # ---- main loop pieces ------------------------------------------
        # fp8 exp outputs land in m-PAIR tiles [P, 2, *] so the column-sum
        # folds run as fp8 DoubleRow ones-matmuls (4 passes per family
        # instead of 8 bf16 passes + 8 DVE adds).
        exabs = {}    # chunk n -> [4 exab pair tiles]
        exaabbs = {}  # chunk n -> [4 exaabb pair tiles]

        def main_chunk(n, at1, at2, ATL1, ATL2):
            h = 512 * (n % 2)
            exabs[n] = []
            exaabbs[n] = []
            for m in range(M_CH):
                a2 = pa2.tile([P, 2 * D], FP32, tag="a2", name=f"a2_{n}_{m}")
                ab = pab.tile([P, D], FP32, tag="ab", name=f"ab_{n}_{m}")
                lo, hi = P * m, P * (m + 1)
                for kp in range(2):
                    ks = slice(2 * kp, 2 * kp + 2)
                    st, sp = kp == 0, kp == 1
                    nc.tensor.matmul(
                        a2[:, 0:D], ATL1[:, ks, lo:hi], at1[:, ks, h : h + 512],
                        start=st, stop=sp, perf_mode=DR,
                    )
                    nc.tensor.matmul(
                        ab, ATL1[:, ks, lo:hi], at2[:, ks, h : h + 512],
                        start=st, stop=sp, perf_mode=DR,
                    )
                for kp in range(2):
                    ks = slice(2 * kp, 2 * kp + 2)
                    nc.tensor.matmul(
                        a2[:, D : 2 * D], ATL2[:, ks, lo:hi],
                        at2[:, ks, h : h + 512],
                        start=(kp == 0), stop=(kp == 1), perf_mode=DR,
                    )
                if m % 2 == 0:
                    exaabb = exa_pool.tile(
                        [P, 2, 2 * D], FP8, tag="exaabb", name=f"exaabb_{n}_{m}"
                    )
                    exab = exp_pool.tile(
                        [P, 2, D], FP8, tag="exab", name=f"exab_{n}_{m}"
                    )
                    exaabbs[n].append(exaabb)
                    exabs[n].append(exab)
                nc.scalar.activation(
                    out=exaabb[:, m % 2, :], in_=a2, func=ACTF.Exp, scale=ESC
                )
                nc.scalar.activation(
                    out=exab[:, m % 2, :], in_=ab, func=ACTF.Exp, scale=ESC,
                    accum_out=rsp_ab[m][:, n : n + 1],
                )

        def fold_chunk(n):
            """Delayed column-sum folds of chunk n: three fp8 DR ones-matmul
            families (ab, aa, bb), 4 pair-passes each; [1,512] results ship
            as small bf16 writes into the AR buffer."""
            colp = psm.tile([16, D], FP32, tag="ps_small", name=f"colp_{n}")
            for mp in range(M_CH // 2):
                nc.tensor.matmul(
                    colp, ones_f8, exabs[n][mp],
                    start=(mp == 0), stop=(mp == M_CH // 2 - 1), perf_mode=DR,
                )
            csb = scratch.tile([1, D], BF16, tag="csb", name=f"csb_{n}", bufs=2)
            nc.vector.tensor_copy(csb, colp[0:1, :])
            if n < 12:
                nc.sync.dma_start(out=cc1_in[0, 512 * n : 512 * (n + 1)], in_=csb)
            else:
                nc.sync.dma_start(
                    out=cc2_in[0, 512 * (n - 12) : 512 * (n - 11)], in_=csb
                )
            for r, half in ((1, 0), (2, 1)):  # aa, bb
                colq = psm.tile([16, D], FP32, tag="ps_small", name=f"colq{r}_{n}")
                for mp in range(M_CH // 2):
                    nc.tensor.matmul(
                        colq, ones_f8,
                        exaabbs[n][mp][:, :, 512 * half : 512 * half + 512],
                        start=(mp == 0), stop=(mp == M_CH // 2 - 1), perf_mode=DR,
                    )
                csq = scratch.tile(
                    [1, D], BF16, tag=f"csq{r}", name=f"csq{r}_{n}", bufs=2
                )
                nc.vector.tensor_copy(csq, colq[0:1, :])
                if n < 12:
                    nc.scalar.dma_start(
                        out=cc1_in[r, 512 * n : 512 * (n + 1)], in_=csq
                    )
                else:
                    nc.scalar.dma_start(
                        out=cc2_in[0, 2048 * r + 512 * (n - 12) : 2048 * r + 512 * (n - 11)],
                        in_=csq,
                    )
            del exabs[n]
            del exaabbs[n]

        def unit_src(u):
            g, off = u // 4, 1024 * (u % 4)
            return (
                SB1[g][:, :, off : off + 1024],
                SB2[g][:, :, off : off + 1024],
            )

        # ---- schedule ---------------------------------------------------
        # all bulk loads enqueue up front (nothing compute-dependent sits on
        # the HWDGE rings); per-unit prep reads superblock slices.
        ZL1, ZL2 = load_sb(z1lT, z2lT, LOCAL, "loc")
        SB1, SB2 = {}, {}
        SB1[0], SB2[0] = load_sb(
            z1T[:, 0:4096], z2T[:, 0:4096], 4096, "g0"
        )
        SB1[1], SB2[1] = load_sb(
            z1T[:, 4096:8192], z2T[:, 4096:8192], 4096, "g1"
        )

        # stage pipeline: s1+s2 run 3 units ahead, s3 two ahead
        p_loc = prep_s1(ZL1, ZL2, "loc")
        prep_s2(p_loc)
        preps = {}
        preps[0] = prep_s1(*unit_src(0), "u0")
        prep_s2(preps[0])
        preps[1] = prep_s1(*unit_src(1), "u1")
        prep_s2(preps[1])
        ATL1, ATL2 = prep_s3(p_loc, pin=True)
        preps[2] = prep_s1(*unit_src(2), "u2")
        prep_s2(preps[2])
        preps[3] = prep_s1(*unit_src(3), "u3")
        prep_s2(preps[3])
        AT1, AT2 = {}, {}
        AT1[0], AT2[0] = prep_s3(preps[0])

        # dab: diag of local a.b product
        for m in range(M_CH):
            dps = psm.tile([P, P], FP32, tag="ps_small", name=f"dps_{m}")
            for kp in range(2):
                nc.tensor.matmul(
                    dps,
                    ATL1[:, 2 * kp : 2 * kp + 2, P * m : P * (m + 1)],
                    ATL2[:, 2 * kp : 2 * kp + 2, P * m : P * (m + 1)],
                    start=(kp == 0), stop=(kp == 1), perf_mode=DR,
                )
            nc.vector.scalar_tensor_tensor(
                out=dtrash, in0=dps, scalar=ISC2, in1=eye_sb,
                op0=ALU.mult, op1=ALU.mult, accum_out=dab[:, m : m + 1],
            )

        # only the GLOBAL sum of dab enters the loss: ship the local sum as
        # one bf16 rider in the small second AR instead of a full row region.
        dsr = small.tile([P, 1], FP32, tag="dsr", name="dsr")
        nc.vector.reduce_sum(out=dsr, in_=dab, axis=X_AX)
        dsp = psm.tile([1, 1], FP32, tag="ps_small", name="dsp")
        nc.tensor.matmul(dsp, ones_f32, dsr, start=True, stop=True)
        dsb = small.tile([1, 1], BF16, tag="dsb", name="dsb")
        nc.vector.tensor_copy(dsb, dsp)
        nc.gpsimd.dma_start(out=cc2_in[0, 7168:7169], in_=dsb)

        AT1[1], AT2[1] = prep_s3(preps[1])
        for u in range(N_UNITS):
            if u % 2 == 1 and u + 3 < N_UNITS:
                for w in (u + 3, u + 4):
                    if w < N_UNITS and w not in preps:
                        preps[w] = prep_s1(*unit_src(w), f"u{w}")
                        prep_s2(preps[w])
            main_chunk(2 * u, AT1[u], AT2[u], ATL1, ATL2)
            if u > 0:
                fold_chunk(2 * u - 1)
            if u == N_UNITS - 2:
                # bulk AR rides under the last four compute chunks
                nc.gpsimd.collective_compute(
                    "AllReduce",
                    ALU.add,
                    replica_groups=[list(range(NCORES))],
                    ins=[cc1_in.opt()],
                    outs=[cc1_out.opt()],
                )
            main_chunk(2 * u + 1, AT1[u], AT2[u], ATL1, ATL2)
            if u + 2 < N_UNITS:
                AT1[u + 2], AT2[u + 2] = prep_s3(preps[u + 2])
            fold_chunk(2 * u)
        fold_chunk(N_CH - 1)

        # ---- tail -------------------------------------------------------
        # local exp(ab) row sums -> masked scatter into global slots
        rs_ab = small.tile([P, M_CH], FP32, tag="rs_ab", name="rs_ab")
        for m in range(M_CH):
            nc.vector.reduce_sum(
                out=rs_ab[:, m : m + 1], in_=rsp_ab[m], axis=X_AX
            )
        rsabm = small.tile([P, N // P], BF16, tag="rsabm", name="rsabm")
        for s in range(NCORES):
            sl = slice(M_CH * s, M_CH * (s + 1))
            nc.vector.tensor_mul(rsabm[:, sl], rs_ab, mask_sb[:, sl])
        nc.gpsimd.dma_start(
            out=cc2_in[1].rearrange("(j p) -> p j", p=P), in_=rsabm
        )

        nc.gpsimd.collective_compute(
            "AllReduce",
            ALU.add,
            replica_groups=[list(range(NCORES))],
            ins=[cc2_in.opt()],
            outs=[cc2_out.opt()],
        )

        # contiguous readback: partition p holds rows 64p..64p+63.  The final
        # math is elementwise + full-sum, so row permutation is irrelevant —
        # it only has to be the SAME permutation for all regions.  cs rows of
        # chunks 12..15 (partitions 96+) come from the small second AR.
        # loss = 0.5/N * (sum ln(d1) + sum ln(d2) - (2/tau)*sum dab); the two
        # ln-sums ride ACT accumulators, sum dab arrives pre-reduced in cc2.
        JW = N // P  # 64 rows per partition
        gt = {}
        for r, nm in enumerate(("cs_ab", "cs_aa", "cs_bb")):
            gt[nm] = scratch.tile([P, JW], BF16, tag=f"g_{nm}", name=f"g_{nm}")
            nc.sync.dma_start(
                out=gt[nm], in_=cc1_out[r].rearrange("(p j) -> p j", p=P)
            )
        gt["rs_ab"] = scratch.tile([P, JW], BF16, tag="g_rs_ab", name="g_rs_ab")
        nc.sync.dma_start(
            out=gt["rs_ab"], in_=cc2_out[1].rearrange("(p j) -> p j", p=P)
        )
        for r, nm in ((0, "cs_ab"), (1, "cs_aa"), (2, "cs_bb")):
            nc.scalar.dma_start(
                out=gt[nm][96:128, :],
                in_=cc2_out[0, 2048 * r : 2048 * (r + 1)].rearrange(
                    "(p j) -> p j", p=32
                ),
            )
        dsg = small.tile([1, 1], BF16, tag="dsg", name="dsg")
        nc.scalar.dma_start(out=dsg, in_=cc2_out[0, 7168:7169])

        # d2 on partitions 0..95 depends only on the first AR -> overlaps AR#2
        denom2 = small.tile([P, JW], FP32, tag="denom2", name="denom2")
        lntr = scratch.tile([P, JW], BF16, tag="lntr", name="lntr")
        s2a = small.tile([P, 1], FP32, tag="s2a", name="s2a")
        nc.vector.memset(s2a, 0.0)
        nc.vector.scalar_tensor_tensor(
            out=denom2[0:96, :], in0=gt["cs_bb"][0:96, :], scalar=-EXPD,
            in1=gt["cs_ab"][0:96, :], op0=ALU.add, op1=ALU.add,
        )
        nc.scalar.activation(
            out=lntr[0:96, :], in_=denom2[0:96, :], func=ACTF.Ln,
            accum_out=s2a[0:96, :],
        )
        s2b = small.tile([P, 1], FP32, tag="s2b", name="s2b")
        nc.vector.memset(s2b, 0.0)
        nc.vector.scalar_tensor_tensor(
            out=denom2[96:128, :], in0=gt["cs_bb"][96:128, :], scalar=-EXPD,
            in1=gt["cs_ab"][96:128, :], op0=ALU.add, op1=ALU.add,
        )
        nc.scalar.activation(
            out=lntr[96:128, :], in_=denom2[96:128, :], func=ACTF.Ln,
            accum_out=s2b[96:128, :],
        )

        denom1 = small.tile([P, JW], FP32, tag="denom1", name="denom1")
        s1 = small.tile([P, 1], FP32, tag="s1", name="s1")
        nc.vector.scalar_tensor_tensor(
            out=denom1, in0=gt["cs_aa"], scalar=-EXPD, in1=gt["rs_ab"],
            op0=ALU.add, op1=ALU.add,
        )
        nc.scalar.activation(
            out=lntr, in_=denom1, func=ACTF.Ln, accum_out=s1
        )

        ssum = small.tile([P, 1], FP32, tag="ssum", name="ssum")
        nc.vector.tensor_add(ssum, s1, s2a)
        nc.vector.tensor_add(ssum, ssum, s2b)
        lps = psm.tile([1, 1], FP32, tag="ps_small", name="lps")
        nc.tensor.matmul(lps, ones_f32, ssum, start=True, stop=True)
        lsb = small.tile([1, 1], FP32, tag="lsb", name="lsb")
        nc.vector.scalar_tensor_tensor(
            out=lsb, in0=dsg, scalar=-2.0 / TAU, in1=lps,
            op0=ALU.mult, op1=ALU.add,
        )
        nc.scalar.mul(lsb, lsb, 0.5 / N)
        nc.scalar.dma_start(out=loss, in_=lsb)

    nc.compile()
    return nc


_NC_CACHE = None


def _get_nc():
    global _NC_CACHE
    if _NC_CACHE is None:
        _NC_CACHE = _build()
    return _NC_CACHE


def _in_maps(z1, z2):
    import ml_dtypes

    z1 = np.ascontiguousarray(np.asarray(z1), dtype=np.float32)
    z2 = np.ascontiguousarray(np.asarray(z2), dtype=np.float32)
    z1T = np.ascontiguousarray(z1.T)
    z2T = np.ascontiguousarray(z2.T)
    z1Tf = z1T.astype(ml_dtypes.float8_e4m3)
    z2Tf = z2T.astype(ml_dtypes.float8_e4m3)
    eye = np.eye(P, dtype=np.float16)
    maps = []
    for c in range(NCORES):
        sl = slice(LOCAL * c, LOCAL * (c + 1))
        rowmask = np.zeros((P, N // P), dtype=np.float32)
        rowmask[:, M_CH * c : M_CH * (c + 1)] = 1.0
        maps.append(
            {
                "z1T": z1Tf,
                "z2T": z2Tf,
                "z1lT": np.ascontiguousarray(z1Tf[:, sl]),
                "z2lT": np.ascontiguousarray(z2Tf[:, sl]),
                "eye": eye,
                "rowmask": rowmask,
            }
        )
    return maps


def kernel(z1, z2):
    nc = _get_nc()
    res = run_bass_kernel_spmd(nc, _in_maps(z1, z2), list(range(NCORES)))
    return np.asarray(res.results[0]["loss"], dtype=np.float32).reshape(())


def kernel_traced(z1, z2):
    """Same as kernel() but with NTFF profiling; returns (loss, exec_time_ns,
    trace_path)."""
    import concourse.bass_utils as bu

    bu.upload_artifacts = lambda tmpdir: "local://" + tmpdir  # no egress
    nc = _get_nc()
    res = run_bass_kernel_spmd(
        nc, _in_maps(z1, z2), list(range(NCORES)), trace=True
    )
    out = np.asarray(res.results[0]["loss"], dtype=np.float32).reshape(())
    trace_path = (
        res.instructions_and_trace[1] if res.instructions_and_trace else None
    )
    return out, res.exec_time_ns, trace_path



# revision 48
# speedup vs baseline: 1.0631x; 1.0631x over previous
"""Contrastive loss (GRACE-style semi_loss pair) on 8 trn2 NeuronCores.

Math (reference):
    a = z1 / ||z1||_row ; b = z2 / ||z2||_row         (N=8192, D=512)
    refl    = exp(a @ a.T / tau) ; between = exp(a @ b.T / tau)
    l1_i = -log(between_ii / (refl.sum(1) + between.sum(1) - refl_ii))
    l2   = same with (z2, z1) swapped
    loss = mean(0.5 * (l1 + l2))

Identities:
  - between2 (for l2) = between.T -> its row sums are COLUMN sums of
    exp(a@b.T/tau).
  - exp(a@a.T) and exp(b@b.T) are symmetric -> their row sums are also
    column sums.  All three column-sum families ride ONE ReduceScatter
    with a [core: ab|aa|bb] interleaved layout; no ACT accumulators or
    DVE row-reduces needed for aa/bb.
  - refl_ii = exp(1/tau) exactly; dab_i = a_i . b_i from fp8 diag blocks.
  - row sumsq (for 1/norm) = diag of the raw z Gram matrix, computed with
    fp8 DoubleRow diag blocks from a casting-DMA fp8 copy of z -- lands
    directly in [128, blocks] layout for a cheap 2-step Newton rsqrt.

Implementation (v4): single pass over zT; fp8e4 DoubleRow matmuls
(K=256/instr, 2x bf16 rate); aa|bb share one 2-bank PSUM tile and a
single [128,1024] exp.  Column sums: exp(ab) via delayed DVE adds,
exp(aa)/exp(bb) via delayed PE ones-matmul folds (one chunk behind so
neither engine waits on ACT).  Prep is stage-pipelined 2-3 units ahead.
Sharding: data-parallel rows; pinned fp8 stationary for the core's 1024
rows, all 16 512-col chunks streamed as moving operands.
"""

import os

# small collectives: RDH has a ~60-120us latency floor here; Mesh is ~10us.
os.environ.setdefault("NEURON_RT_DBG_RDH_CC", "0")

import numpy as np
from contextlib import ExitStack

KDEBUG = bool(os.environ.get("KDEBUG"))

import concourse.bass as bass
import concourse.tile as tile
from concourse import bacc, mybir
from concourse.bass_utils import run_bass_kernel_spmd

N = 8192
D = 512
P = 128
NCORES = 8
LOCAL = N // NCORES            # 1024 rows per core
M_CH = LOCAL // P              # 8 local row blocks of 128
N_UNITS = 8                    # 1024-column units
N_CH = 16                      # 512-column chunks
KC = D // P                    # 4 contraction chunks of 128
TAU = 0.4
SC = 16.0                      # fp8 operand scale: a~N(0,1/512) -> sigma .71
ESC = 1.0 / (SC * SC * TAU)    # exp() scale folding fp8 scaling + 1/tau
ISC2 = 1.0 / (SC * SC)
EXPD = float(np.exp(1.0 / TAU))
Y0 = float(D) ** -0.5          # Newton rsqrt seed; sumsq ~ 512 +- 6%

FP32 = mybir.dt.float32
BF16 = mybir.dt.bfloat16
FP16 = mybir.dt.float16
FP8 = mybir.dt.float8e4
ALU = mybir.AluOpType
ACTF = mybir.ActivationFunctionType
DR = mybir.MatmulPerfMode.DoubleRow
X_AX = mybir.AxisListType.X


def _build():
    nc = bacc.Bacc("TRN2", debug=False, num_devices=NCORES)
    # fp8 copies are prepared host-side so every device load is a plain
    # (non-cast) DMA; the operand pipeline quantizes to fp8 anyway, so this
    # only stacks a second tiny rounding on the same quantization step.
    z1T = nc.dram_tensor("z1T", [D, N], FP8, kind="ExternalInput").ap()
    z2T = nc.dram_tensor("z2T", [D, N], FP8, kind="ExternalInput").ap()
    z1lT = nc.dram_tensor("z1lT", [D, LOCAL], FP8, kind="ExternalInput").ap()
    z2lT = nc.dram_tensor("z2lT", [D, LOCAL], FP8, kind="ExternalInput").ap()
    eye = nc.dram_tensor("eye", [P, P], FP16, kind="ExternalInput").ap()
    # per-core one-hot row mask: rowmask[p, s*M_CH+m] = 1 iff slot s == core id
    rowmask = nc.dram_tensor("rowmask", [P, N // P], FP32, kind="ExternalInput").ap()
    loss = nc.dram_tensor("loss", [1, 1], FP32, kind="ExternalOutput").ap()
    if KDEBUG:
        dbg = {
            nm: nc.dram_tensor(f"dbg_{nm}", [P, N // P], FP32, kind="ExternalOutput").ap()
            for nm in ("cs_ab", "cs_aa", "cs_bb", "dab", "rs_ab", "d1", "d2")
        }

    with tile.TileContext(nc) as tc, ExitStack() as ctx:
        big = ctx.enter_context(tc.tile_pool(name="big", bufs=1))
        zst = ctx.enter_context(tc.tile_pool(name="zst", bufs=3))
        zqt = ctx.enter_context(tc.tile_pool(name="zqt", bufs=2))
        atp = ctx.enter_context(tc.tile_pool(name="atp", bufs=3))
        small = ctx.enter_context(tc.tile_pool(name="small", bufs=1))
        scratch = ctx.enter_context(tc.tile_pool(name="scratch", bufs=2))
        exa_pool = ctx.enter_context(tc.tile_pool(name="exa_pool", bufs=10))
        exp_pool = ctx.enter_context(tc.tile_pool(name="exp_pool", bufs=10))
        pa2 = ctx.enter_context(tc.tile_pool(name="pa2", bufs=2, space="PSUM"))
        pab = ctx.enter_context(tc.tile_pool(name="pab", bufs=2, space="PSUM"))
        psm = ctx.enter_context(tc.tile_pool(name="psm", bufs=2, space="PSUM"))
        dram = ctx.enter_context(tc.tile_pool(name="dram", bufs=1, space="DRAM"))

        # ---- constants --------------------------------------------------
        # DR stationary needs k-step %16==0 -> 16 ones columns (row 0 used)
        ones_f8 = small.tile([P, 2, 16], FP8, tag="ones_f8", name="ones_f8")
        nc.vector.memset(ones_f8, 1.0)
        ones_f32 = small.tile([P, 1], FP32, tag="ones_f32", name="ones_f32")
        nc.vector.memset(ones_f32, 1.0)
        eye_sb = small.tile([P, P], FP16, tag="eye", name="eye_sb")
        nc.sync.dma_start(out=eye_sb, in_=eye)

        # ---- persistent -------------------------------------------------
        dab = small.tile([P, M_CH], FP32, tag="dab", name="dab")
        rsp_ab = [
            small.tile([P, N_CH], FP32, tag=f"rsp_ab{m}", name=f"rsp_ab{m}")
            for m in range(M_CH)
        ]
        dtrash = small.tile([P, P], BF16, tag="dtrash", name="dtrash")

        # fused AllReduce, bf16, split in two so the bulk rides under the
        # last two compute chunks and only a small AR sits on the tail:
        #  cc1 [4, N]: colsum exp(ab|aa|bb) chunks 0..13 + dab (masked rows)
        #  cc2 [2, N]: row0 = colsum chunks 14,15 packed [r*1024 + (n-14)*512],
        #              row1 = rowsum exp(ab) (masked rows)
        # After both ARs every core holds all global sums and computes the
        # full scalar loss redundantly -> no trailing scalar collective.
        cc1_in = dram.tile([3, N], BF16, tag="cc1_in", name="cc1_in")
        cc1_out = dram.tile(
            [3, N], BF16, tag="cc1_out", name="cc1_out", addr_space="Shared"
        )
        cc2_in = dram.tile([2, N], BF16, tag="cc2_in", name="cc2_in")
        cc2_out = dram.tile(
            [2, N], BF16, tag="cc2_out", name="cc2_out", addr_space="Shared"
        )
        mask_sb = small.tile([P, N // P], FP32, tag="mask_sb", name="mask_sb")
        nc.sync.dma_start(out=mask_sb, in_=rowmask)

        # ---- unit prep (staged) ----------------------------------------
        # s1: casting loads (bf16 + raw fp8), Gram-diag sumsq
        # s2: Newton rsqrt, broadcast round-trip
        # s3: fp8 operand scaling
        class Prep:
            pass

        def load_sb(src1, src2, width, name):
            """One fp8 superblock load per tensor: [P, KC, width] with
            width*1B descriptors per (p, k) line -- packet-rate friendly."""
            t1 = zst.tile([P, KC, width], FP8, tag=f"sb1_{name}",
                          name=f"sb1_{name}", bufs=1)
            nc.sync.dma_start(
                out=t1, in_=src1.rearrange("(k p) j -> p k j", p=P)
            )
            t2 = zst.tile([P, KC, width], FP8, tag=f"sb2_{name}",
                          name=f"sb2_{name}", bufs=1)
            nc.scalar.dma_start(
                out=t2, in_=src2.rearrange("(k p) j -> p k j", p=P)
            )
            return t1, t2

        def prep_s1(zs1, zs2, name):
            p_ = Prep()
            p_.name = name
            p_.zs1 = zs1
            p_.zs2 = zs2
            # column sumsq via ones-DR-matmul over an fp8 squared copy, then
            # ACT Rsqrt straight out of PSUM (SC fold: rsqrt(ss/SC^2)=SC/√ss)
            p_.rl = scratch.tile([1, 2048], BF16, tag="rl", name=f"rl_{name}",
                                 bufs=3)
            # rsqrt via exp(-0.5*ln(x)) with exactly ONE Ln and ONE Exp
            # per unit: Ln and Exp live in different ACT tables, and every
            # Ln instruction costs two table swaps around it -- so the four
            # sumsq quarters are first gathered into one staging tile.
            stg = scratch.tile(
                [1, 2048], BF16, tag="stg", name=f"stg_{name}", bufs=2
            )
            for t, zs in ((0, p_.zs1), (1, p_.zs2)):
                zq = zqt.tile([P, KC, 1024], FP8, tag="zq", name=f"zq_{name}_{t}")
                nc.vector.tensor_mul(zq, zs, zs)
                for hf in range(2):
                    sq = psm.tile([16, D], FP32, tag="ps_small",
                                  name=f"sq_{name}_{t}_{hf}")
                    for kp in range(2):
                        nc.tensor.matmul(
                            sq, ones_f8,
                            zq[:, 2 * kp : 2 * kp + 2, 512 * hf : 512 * hf + 512],
                            start=(kp == 0), stop=(kp == 1), perf_mode=DR,
                        )
                    o = 1024 * t + 512 * hf
                    nc.vector.tensor_copy(stg[:, o : o + 512], sq[0:1, :])
            lnt = scratch.tile(
                [1, 2048], FP16, tag="lnt", name=f"lnt_{name}", bufs=2
            )
            nc.scalar.activation(
                out=lnt, in_=stg, func=ACTF.Ln, scale=1.0 / (SC * SC)
            )
            nc.scalar.activation(
                out=p_.rl, in_=lnt, func=ACTF.Exp, scale=-0.5
            )
            return p_

        def prep_s2(p_):
            name = p_.name
            rl_d = dram.tile([1, 2048], BF16, tag=f"rld_{name}", name=f"rld_{name}")
            nc.gpsimd.dma_start(out=rl_d, in_=p_.rl)
            rb = scratch.tile(
                [P, 2048], BF16, tag="rb", name=f"rb_{name}", bufs=3
            )
            nc.gpsimd.dma_start(
                out=rb, in_=rl_d.to_broadcast([P, 2048])
            )
            p_.rb1 = rb[:, 0:1024]
            p_.rb2 = rb[:, 1024:2048]

        def prep_s3(p_, pin=False):
            name = p_.name
            if pin:
                at1 = big.tile([P, KC, 1024], FP8, tag="ATL1", name="ATL1")
                at2 = big.tile([P, KC, 1024], FP8, tag="ATL2", name="ATL2")
            else:
                at1 = atp.tile([P, KC, 1024], FP8, tag="at1", name=f"at1_{name}")
                at2 = atp.tile([P, KC, 1024], FP8, tag="at2", name=f"at2_{name}")
            for k in range(KC):
                eng = nc.gpsimd if k == 0 else nc.vector
                eng.tensor_mul(at1[:, k, :], p_.zs1[:, k, :], p_.rb1)
                eng = nc.gpsimd if k == 3 else nc.vector
                eng.tensor_mul(at2[:, k, :], p_.zs2[:, k, :], p_.rb2)
            return at1, at2

        # ---- main loop pieces ------------------------------------------
        # fp8 exp outputs land in m-PAIR tiles [P, 2, *] so the column-sum
        # folds run as fp8 DoubleRow ones-matmuls (4 passes per family
        # instead of 8 bf16 passes + 8 DVE adds).
        exabs = {}    # chunk n -> [4 exab pair tiles]
        exaabbs = {}  # chunk n -> [4 exaabb pair tiles]

        def main_chunk(n, at1, at2, ATL1, ATL2):
            h = 512 * (n % 2)
            exabs[n] = []
            exaabbs[n] = []
            for m in range(M_CH):
                a2 = pa2.tile([P, 2 * D], FP32, tag="a2", name=f"a2_{n}_{m}")
                ab = pab.tile([P, D], FP32, tag="ab", name=f"ab_{n}_{m}")
                lo, hi = P * m, P * (m + 1)
                for kp in range(2):
                    ks = slice(2 * kp, 2 * kp + 2)
                    st, sp = kp == 0, kp == 1
                    nc.tensor.matmul(
                        a2[:, 0:D], ATL1[:, ks, lo:hi], at1[:, ks, h : h + 512],
                        start=st, stop=sp, perf_mode=DR,
                    )
                    nc.tensor.matmul(
                        ab, ATL1[:, ks, lo:hi], at2[:, ks, h : h + 512],
                        start=st, stop=sp, perf_mode=DR,
                    )
                for kp in range(2):
                    ks = slice(2 * kp, 2 * kp + 2)
                    nc.tensor.matmul(
                        a2[:, D : 2 * D], ATL2[:, ks, lo:hi],
                        at2[:, ks, h : h + 512],
                        start=(kp == 0), stop=(kp == 1), perf_mode=DR,
                    )
                if m % 2 == 0:
                    exaabb = exa_pool.tile(
                        [P, 2, 2 * D], FP8, tag="exaabb", name=f"exaabb_{n}_{m}"
                    )
                    exab = exp_pool.tile(
                        [P, 2, D], FP8, tag="exab", name=f"exab_{n}_{m}"
                    )
                    exaabbs[n].append(exaabb)
                    exabs[n].append(exab)
                nc.scalar.activation(
                    out=exaabb[:, m % 2, :], in_=a2, func=ACTF.Exp, scale=ESC
                )
                nc.scalar.activation(
                    out=exab[:, m % 2, :], in_=ab, func=ACTF.Exp, scale=ESC,
                    accum_out=rsp_ab[m][:, n : n + 1],
                )

        def fold_chunk(n):
            """Delayed column-sum folds of chunk n: three fp8 DR ones-matmul
            families (ab, aa, bb), 4 pair-passes each; [1,512] results ship
            as small bf16 writes into the AR buffer."""
            colp = psm.tile([16, D], FP32, tag="ps_small", name=f"colp_{n}")
            for mp in range(M_CH // 2):
                nc.tensor.matmul(
                    colp, ones_f8, exabs[n][mp],
                    start=(mp == 0), stop=(mp == M_CH // 2 - 1), perf_mode=DR,
                )
            csb = scratch.tile([1, D], BF16, tag="csb", name=f"csb_{n}", bufs=2)
            nc.vector.tensor_copy(csb, colp[0:1, :])
            if n < 12:
                nc.sync.dma_start(out=cc1_in[0, 512 * n : 512 * (n + 1)], in_=csb)
            else:
                nc.sync.dma_start(
                    out=cc2_in[0, 512 * (n - 12) : 512 * (n - 11)], in_=csb
                )
            for r, half in ((1, 0), (2, 1)):  # aa, bb
                colq = psm.tile([16, D], FP32, tag="ps_small", name=f"colq{r}_{n}")
                for mp in range(M_CH // 2):
                    nc.tensor.matmul(
                        colq, ones_f8,
                        exaabbs[n][mp][:, :, 512 * half : 512 * half + 512],
                        start=(mp == 0), stop=(mp == M_CH // 2 - 1), perf_mode=DR,
                    )
                csq = scratch.tile(
                    [1, D], BF16, tag=f"csq{r}", name=f"csq{r}_{n}", bufs=2
                )
                nc.vector.tensor_copy(csq, colq[0:1, :])
                if n < 12:
                    nc.scalar.dma_start(
                        out=cc1_in[r, 512 * n : 512 * (n + 1)], in_=csq
                    )
                else:
                    nc.scalar.dma_start(
                        out=cc2_in[0, 2048 * r + 512 * (n - 12) : 2048 * r + 512 * (n - 11)],
                        in_=csq,
                    )
            del exabs[n]
            del exaabbs[n]

        def unit_src(u):
            g, off = u // 4, 1024 * (u % 4)
            return (
                SB1[g][:, :, off : off + 1024],
                SB2[g][:, :, off : off + 1024],
            )

        # ---- schedule ---------------------------------------------------
        # all bulk loads enqueue up front (nothing compute-dependent sits on
        # the HWDGE rings); per-unit prep reads superblock slices.
        ZL1, ZL2 = load_sb(z1lT, z2lT, LOCAL, "loc")
        SB1, SB2 = {}, {}
        SB1[0], SB2[0] = load_sb(
            z1T[:, 0:4096], z2T[:, 0:4096], 4096, "g0"
        )
        SB1[1], SB2[1] = load_sb(
            z1T[:, 4096:8192], z2T[:, 4096:8192], 4096, "g1"
        )

        # stage pipeline: s1+s2 run 3 units ahead, s3 two ahead
        p_loc = prep_s1(ZL1, ZL2, "loc")
        prep_s2(p_loc)
        preps = {}
        preps[0] = prep_s1(*unit_src(0), "u0")
        prep_s2(preps[0])
        preps[1] = prep_s1(*unit_src(1), "u1")
        prep_s2(preps[1])
        ATL1, ATL2 = prep_s3(p_loc, pin=True)
        preps[2] = prep_s1(*unit_src(2), "u2")
        prep_s2(preps[2])
        preps[3] = prep_s1(*unit_src(3), "u3")
        prep_s2(preps[3])
        AT1, AT2 = {}, {}
        AT1[0], AT2[0] = prep_s3(preps[0])

        # dab: diag of local a.b product
        for m in range(M_CH):
            dps = psm.tile([P, P], FP32, tag="ps_small", name=f"dps_{m}")
            for kp in range(2):
                nc.tensor.matmul(
                    dps,
                    ATL1[:, 2 * kp : 2 * kp + 2, P * m : P * (m + 1)],
                    ATL2[:, 2 * kp : 2 * kp + 2, P * m : P * (m + 1)],
                    start=(kp == 0), stop=(kp == 1), perf_mode=DR,
                )
            nc.vector.scalar_tensor_tensor(
                out=dtrash, in0=dps, scalar=ISC2, in1=eye_sb,
                op0=ALU.mult, op1=ALU.mult, accum_out=dab[:, m : m + 1],
            )

        # only the GLOBAL sum of dab enters the loss: ship the local sum as
        # one bf16 rider in the small second AR instead of a full row region.
        dsr = small.tile([P, 1], FP32, tag="dsr", name="dsr")
        nc.vector.reduce_sum(out=dsr, in_=dab, axis=X_AX)
        dsp = psm.tile([1, 1], FP32, tag="ps_small", name="dsp")
        nc.tensor.matmul(dsp, ones_f32, dsr, start=True, stop=True)
        dsb = small.tile([1, 1], BF16, tag="dsb", name="dsb")
        nc.vector.tensor_copy(dsb, dsp)
        nc.gpsimd.dma_start(out=cc2_in[0, 7168:7169], in_=dsb)

        AT1[1], AT2[1] = prep_s3(preps[1])
        for u in range(N_UNITS):
            if u % 2 == 1 and u + 3 < N_UNITS:
                for w in (u + 3, u + 4):
                    if w < N_UNITS and w not in preps:
                        preps[w] = prep_s1(*unit_src(w), f"u{w}")
                        prep_s2(preps[w])
            main_chunk(2 * u, AT1[u], AT2[u], ATL1, ATL2)
            if u > 0:
                fold_chunk(2 * u - 1)
            if u == N_UNITS - 2:
                # bulk AR rides under the last four compute chunks
                nc.gpsimd.collective_compute(
                    "AllReduce",
                    ALU.add,
                    replica_groups=[list(range(NCORES))],
                    ins=[cc1_in.opt()],
                    outs=[cc1_out.opt()],
                )
            main_chunk(2 * u + 1, AT1[u], AT2[u], ATL1, ATL2)
            if u + 2 < N_UNITS:
                AT1[u + 2], AT2[u + 2] = prep_s3(preps[u + 2])
            fold_chunk(2 * u)
        fold_chunk(N_CH - 1)

        # ---- tail -------------------------------------------------------
        # local exp(ab) row sums -> masked scatter into global slots
        rs_ab = small.tile([P, M_CH], FP32, tag="rs_ab", name="rs_ab")
        for m in range(M_CH):
            nc.vector.reduce_sum(
                out=rs_ab[:, m : m + 1], in_=rsp_ab[m], axis=X_AX
            )
        rsabm = small.tile([P, N // P], BF16, tag="rsabm", name="rsabm")
        for s in range(NCORES):
            sl = slice(M_CH * s, M_CH * (s + 1))
            nc.vector.tensor_mul(rsabm[:, sl], rs_ab, mask_sb[:, sl])
        nc.gpsimd.dma_start(
            out=cc2_in[1].rearrange("(j p) -> p j", p=P), in_=rsabm
        )

        nc.gpsimd.collective_compute(
            "AllReduce",
            ALU.add,
            replica_groups=[list(range(NCORES))],
            ins=[cc2_in.opt()],
            outs=[cc2_out.opt()],
        )

        # contiguous readback: partition p holds rows 64p..64p+63.  The final
        # math is elementwise + full-sum, so row permutation is irrelevant —
        # it only has to be the SAME permutation for all regions.  cs rows of
        # chunks 12..15 (partitions 96+) come from the small second AR.
        # loss = 0.5/N * (sum ln(d1) + sum ln(d2) - (2/tau)*sum dab); the two
        # ln-sums ride ACT accumulators, sum dab arrives pre-reduced in cc2.
        JW = N // P  # 64 rows per partition
        gt = {}
        for r, nm in enumerate(("cs_ab", "cs_aa", "cs_bb")):
            gt[nm] = scratch.tile([P, JW], BF16, tag=f"g_{nm}", name=f"g_{nm}")
            nc.sync.dma_start(
                out=gt[nm], in_=cc1_out[r].rearrange("(p j) -> p j", p=P)
            )
        gt["rs_ab"] = scratch.tile([P, JW], BF16, tag="g_rs_ab", name="g_rs_ab")
        nc.sync.dma_start(
            out=gt["rs_ab"], in_=cc2_out[1].rearrange("(p j) -> p j", p=P)
        )
        for r, nm in ((0, "cs_ab"), (1, "cs_aa"), (2, "cs_bb")):
            nc.scalar.dma_start(
                out=gt[nm][96:128, :],
                in_=cc2_out[0, 2048 * r : 2048 * (r + 1)].rearrange(
                    "(p j) -> p j", p=32
                ),
            )
        dsg = small.tile([1, 1], BF16, tag="dsg", name="dsg")
        nc.scalar.dma_start(out=dsg, in_=cc2_out[0, 7168:7169])

        # d2 on partitions 0..95 depends only on the first AR -> overlaps AR#2
        denom2 = small.tile([P, JW], FP32, tag="denom2", name="denom2")
        lntr = scratch.tile([P, JW], BF16, tag="lntr", name="lntr")
        s2a = small.tile([P, 1], FP32, tag="s2a", name="s2a")
        nc.vector.memset(s2a, 0.0)
        nc.vector.scalar_tensor_tensor(
            out=denom2[0:96, :], in0=gt["cs_bb"][0:96, :], scalar=-EXPD,
            in1=gt["cs_ab"][0:96, :], op0=ALU.add, op1=ALU.add,
        )
        nc.scalar.activation(
            out=lntr[0:96, :], in_=denom2[0:96, :], func=ACTF.Ln,
            accum_out=s2a[0:96, :],
        )
        s2b = small.tile([P, 1], FP32, tag="s2b", name="s2b")
        nc.vector.memset(s2b, 0.0)
        nc.vector.scalar_tensor_tensor(
            out=denom2[96:128, :], in0=gt["cs_bb"][96:128, :], scalar=-EXPD,
            in1=gt["cs_ab"][96:128, :], op0=ALU.add, op1=ALU.add,
        )
        nc.scalar.activation(
            out=lntr[96:128, :], in_=denom2[96:128, :], func=ACTF.Ln,
            accum_out=s2b[96:128, :],
        )

        denom1 = small.tile([P, JW], FP32, tag="denom1", name="denom1")
        s1 = small.tile([P, 1], FP32, tag="s1", name="s1")
        nc.vector.scalar_tensor_tensor(
            out=denom1, in0=gt["cs_aa"], scalar=-EXPD, in1=gt["rs_ab"],
            op0=ALU.add, op1=ALU.add,
        )
        nc.scalar.activation(
            out=lntr, in_=denom1, func=ACTF.Ln, accum_out=s1
        )

        ssum = small.tile([P, 1], FP32, tag="ssum", name="ssum")
        nc.vector.tensor_add(ssum, s1, s2a)
        nc.vector.tensor_add(ssum, ssum, s2b)
        lps = psm.tile([1, 1], FP32, tag="ps_small", name="lps")
        nc.tensor.matmul(lps, ones_f32, ssum, start=True, stop=True)
        lsb = small.tile([1, 1], FP32, tag="lsb", name="lsb")
        nc.vector.scalar_tensor_tensor(
            out=lsb, in0=dsg, scalar=-2.0 / TAU, in1=lps,
            op0=ALU.mult, op1=ALU.add,
        )
        nc.scalar.mul(lsb, lsb, 0.5 / N)
        nc.scalar.dma_start(out=loss, in_=lsb)

    nc.compile()
    return nc


_NC_CACHE = None


def _get_nc():
    global _NC_CACHE
    if _NC_CACHE is None:
        _NC_CACHE = _build()
    return _NC_CACHE


def _in_maps(z1, z2):
    import ml_dtypes

    z1 = np.ascontiguousarray(np.asarray(z1), dtype=np.float32)
    z2 = np.ascontiguousarray(np.asarray(z2), dtype=np.float32)
    z1T = np.ascontiguousarray(z1.T)
    z2T = np.ascontiguousarray(z2.T)
    z1Tf = z1T.astype(ml_dtypes.float8_e4m3)
    z2Tf = z2T.astype(ml_dtypes.float8_e4m3)
    eye = np.eye(P, dtype=np.float16)
    maps = []
    for c in range(NCORES):
        sl = slice(LOCAL * c, LOCAL * (c + 1))
        rowmask = np.zeros((P, N // P), dtype=np.float32)
        rowmask[:, M_CH * c : M_CH * (c + 1)] = 1.0
        maps.append(
            {
                "z1T": z1Tf,
                "z2T": z2Tf,
                "z1lT": np.ascontiguousarray(z1Tf[:, sl]),
                "z2lT": np.ascontiguousarray(z2Tf[:, sl]),
                "eye": eye,
                "rowmask": rowmask,
            }
        )
    return maps


def kernel(z1, z2):
    nc = _get_nc()
    res = run_bass_kernel_spmd(nc, _in_maps(z1, z2), list(range(NCORES)))
    return np.asarray(res.results[0]["loss"], dtype=np.float32).reshape(())


def kernel_traced(z1, z2):
    """Same as kernel() but with NTFF profiling; returns (loss, exec_time_ns,
    trace_path)."""
    import concourse.bass_utils as bu

    bu.upload_artifacts = lambda tmpdir: "local://" + tmpdir  # no egress
    nc = _get_nc()
    res = run_bass_kernel_spmd(
        nc, _in_maps(z1, z2), list(range(NCORES)), trace=True
    )
    out = np.asarray(res.results[0]["loss"], dtype=np.float32).reshape(())
    trace_path = (
        res.instructions_and_trace[1] if res.instructions_and_trace else None
    )
    return out, res.exec_time_ns, trace_path

